# revision 22
# baseline (speedup 1.0000x reference)
"""Trainium2 Bass kernel for nn_DroneNoiseGAT (3-layer GAT + head MLP).

Sharding: 8 cores; core c handles batch b=c//4, destination-row block
rb=c%4 (512 rows of the 2048-node graph). Each core computes its rows'
attention (all three layers) against the full node set; the per-layer
node features needed by every core are exchanged with AllGathers over
4-core replica groups at each layer boundary.

Key algebra: with leaky-relu slope 0.2,
    exp(lrelu(s_i + d_j)) = max(exp(s_i)exp(d_j), exp(.2 s_i)exp(.2 d_j))
                          = exp(.2 s_i) exp(d_j) max(r_i, q_j)
with r = exp(.8 s), q = exp(-.8 d). The per-destination factor
exp(.2 s_i) cancels between softmax numerator and denominator, and
exp(d_j) folds into the staged per-node features (Wh*F and F in place
of Wh and 1). The whole NxN attention map therefore costs ONE fused
DVE op per 128x512 tile:  alpha = (r_bcast max q_col) * adj,
followed by the aggregation matmul [WhF|F]^T @ alpha whose last row is
the softmax denominator.

A tiny warmup AllGather is issued at kernel start so the collective
ring's one-time setup cost is absorbed under layer-1 compute.
"""

from contextlib import ExitStack

import numpy as np
import ml_dtypes

import concourse.bass as bass
import concourse.bacc as bacc
import concourse.mybir as mybir
import concourse.tile as tile
from concourse.masks import make_identity

BF = mybir.dt.bfloat16
F32 = mybir.dt.float32
AF = mybir.ActivationFunctionType
ALU = mybir.AluOpType

bf16 = ml_dtypes.bfloat16

# problem constants
B, N, IN, HID, H = 2, 2048, 32, 64, 4
D = H * HID
NEG_SLOPE = 0.2
LN_EPS = 1e-5

P = 128
N_CORES = 8

# staging layout (bf16): per-head blocks [Wh_h*F_h | F_h] of 65 cols
# ([0:260]), then q_h = exp(-.8 d_h) in [260:264]
SC = 264


class Cfg:
    """Geometry + engine-balance knobs."""

    def __init__(self, n=N, ni=None, gp_heads=(3,), warmup_cc=True,
                 debug=False, fake_cc=False, bufs=None, stop_after=99):
        self.stop_after = stop_after
        self.bufs = dict(tmp=2, alphap=8, smallp=4, stp=4, whgp=2, egp=1,
                         ps_sm=4)
        if bufs:
            self.bufs.update(bufs)
        self.debug = debug
        self.fake_cc = fake_cc  # replace AllGather with local DMAs
        self.warmup_cc = warmup_cc and not fake_cc
        self.n = n                      # total nodes
        self.ni = ni or (n * B // N_CORES)  # own destination rows
        self.njt = n // P               # j tiles
        self.nit = self.ni // P         # own i tiles
        assert n % P == 0 and self.ni % P == 0
        self.gp_heads = set(gp_heads)   # heads whose alpha op runs on gpsimd


def build_nc(cfg: Cfg, n_cores=N_CORES, groups=None):
    nc = bacc.Bacc(num_devices=n_cores)
    groups = groups or [
        list(range(g * 4, g * 4 + 4)) for g in range(max(1, n_cores // 4))
    ]
    n, ni, njt, nit = cfg.n, cfg.ni, cfg.njt, cfg.nit

    # ---- DRAM I/O ----
    madj = nc.dram_tensor("madj", [njt, P, ni], BF, kind="ExternalInput")
    stage1 = nc.dram_tensor("stage1", [njt, P, SC], BF, kind="ExternalInput")
    eg1 = nc.dram_tensor("eg1", [4, ni], BF, kind="ExternalInput")
    xs1 = nc.dram_tensor("xs1", [ni, D], F32, kind="ExternalInput")
    w2 = nc.dram_tensor("w2", [2, P, D], BF, kind="ExternalInput")
    w3 = nc.dram_tensor("w3", [2, P, D], BF, kind="ExternalInput")
    skip3 = nc.dram_tensor("skip3", [2, P, HID], BF, kind="ExternalInput")
    asd2 = nc.dram_tensor("asd2", [2, P, 8], BF, kind="ExternalInput")
    asd3 = nc.dram_tensor("asd3", [2, P, 8], BF, kind="ExternalInput")
    hmlp1 = nc.dram_tensor("hmlp1", [HID + 1, 32], BF, kind="ExternalInput")
    hmlp2 = nc.dram_tensor("hmlp2", [33, 1], BF, kind="ExternalInput")
    hb1c = nc.dram_tensor("hb1c", [32, 1], BF, kind="ExternalInput")
    out_d = nc.dram_tensor("out", [ni, 1], F32, kind="ExternalOutput")
    if cfg.debug:
        dbg_h1 = nc.dram_tensor("dbg_h1", [ni, D], F32, kind="ExternalOutput")
        dbg_h2 = nc.dram_tensor("dbg_h2", [ni, D], F32, kind="ExternalOutput")
        dbg_h3 = nc.dram_tensor("dbg_h3", [ni, HID], F32, kind="ExternalOutput")

    cc_in = nc.dram_tensor("cc_in", [ni, SC], BF)
    nh = ni // 2
    cc_out = [nc.dram_tensor(f"cc_out{hf}", [4 * nh, SC], BF)
              for hf in range(2)]
    if cfg.warmup_cc:
        # full-size mirror of a half-gather: the ring's first transfer of a
        # given size class runs ~6x below steady-state bandwidth, so pay
        # that cost on dummy data concurrently with layer-1 attention
        wu_in = nc.dram_tensor("wu_in", [nh, SC], BF)
        wu_out = nc.dram_tensor("wu_out", [4 * nh, SC], BF)

    with tile.TileContext(nc) as tc, ExitStack() as ctx:
        consts = ctx.enter_context(tc.tile_pool(name="consts", bufs=1))
        adjp = ctx.enter_context(tc.tile_pool(name="adjp", bufs=1))
        bu = cfg.bufs
        whgp = ctx.enter_context(tc.tile_pool(name="whgp", bufs=bu["whgp"]))
        egp = ctx.enter_context(tc.tile_pool(name="egp", bufs=bu["egp"]))
        hp = ctx.enter_context(tc.tile_pool(name="hp", bufs=1))
        tmp = ctx.enter_context(tc.tile_pool(name="tmp", bufs=bu["tmp"]))
        alphap = ctx.enter_context(tc.tile_pool(name="alphap", bufs=bu["alphap"]))
        smallp = ctx.enter_context(tc.tile_pool(name="smallp", bufs=bu["smallp"]))
        stp = ctx.enter_context(tc.tile_pool(name="stp", bufs=bu["stp"]))
        psum_agg = ctx.enter_context(tc.tile_pool(name="psA", bufs=1, space="PSUM"))
        psum_sm = ctx.enter_context(
            tc.tile_pool(name="psS", bufs=bu["ps_sm"], space="PSUM"))

        # warmup collective: first CC op on the ring pays a large one-time
        # setup cost; pay it on 128 bytes concurrently with layer-1 compute
        # instead of on the 139KB layer-boundary gather
        if cfg.warmup_cc:
            nc.gpsimd.collective_compute(
                "AllGather", ALU.bypass, replica_groups=groups,
                ins=[wu_in[:]], outs=[wu_out[:]])

        # ---- constants ----
        ident_bf = consts.tile([P, P], BF)
        make_identity(nc, ident_bf)
        ident_f = consts.tile([P, P], F32)
        make_identity(nc, ident_f)
        eps_sb = consts.tile([P, 1], F32)
        nc.vector.memset(eps_sb, LN_EPS)
        w_sb = {l: [consts.tile([P, D], BF, name=f"w{l}s{kt}") for kt in range(2)]
                for l in (2, 3)}
        asd_sb = {l: [consts.tile([P, 8], BF, name=f"asd{l}s{kt}")
                      for kt in range(2)] for l in (2, 3)}
        skip3_sb = [consts.tile([P, HID], BF, name=f"sk3s{kt}") for kt in range(2)]
        hmlp1_sb = consts.tile([HID + 1, 32], BF)
        hmlp2_sb = consts.tile([33, 1], BF)
        hb1c_sb = consts.tile([32, 1], BF)

        def load_late_consts():
            # weights needed only from stage_W onward; emitting their DMAs
            # after L1's attention keeps the startup DMA queues for the
            # tensors that gate the first alpha tiles
            for kt in range(2):
                nc.sync.dma_start(out=w_sb[2][kt], in_=w2[kt])
                nc.sync.dma_start(out=w_sb[3][kt], in_=w3[kt])
                nc.sync.dma_start(out=asd_sb[2][kt], in_=asd2[kt])
                nc.sync.dma_start(out=asd_sb[3][kt], in_=asd3[kt])
                nc.sync.dma_start(out=skip3_sb[kt], in_=skip3[kt])
            nc.sync.dma_start(out=hmlp1_sb, in_=hmlp1[:])
            nc.sync.dma_start(out=hmlp2_sb, in_=hmlp2[:])
            nc.sync.dma_start(out=hb1c_sb, in_=hb1c[:])
        ones1 = consts.tile([1, P], BF)
        nc.vector.memset(ones1, 1.0)

        # adjacency, resident all layers; DMAs emitted jt-interleaved with
        # the layer-1 whg loads so tile jt=0 starts attention without
        # waiting behind the full 2MB adjacency load
        madj_sb = [adjp.tile([P, ni], BF, name=f"madj{jt}") for jt in range(njt)]

        # ============ per-layer machinery ============

        def load_layer_inputs(layer, src_whg, egT_rows):
            """Load/prepare: Whg tiles, q f32 extracts, r broadcast.

            src_whg(jt) -> DRAM AP [P, SC]; egT_rows: [4, ni] bf16 rows
            (SBUF tile or DRAM AP) holding r = exp(.8 s) for own i.
            """
            egb = egp.tile([P, 4, ni], BF, name="egb", tag="egb")
            for r in range(4):
                # rank-1 PE broadcast (ones x row): SBUF rows can't be
                # partition-broadcast by DMA. PE needs the row at
                # partition 0: hop it there by DMA.
                egr = smallp.tile([1, ni], BF, name="egr", tag="egr")
                nc.sync.dma_start(out=egr, in_=egT_rows[r:r + 1, :])
                bp = psum_sm.tile([P, ni], F32, name="bcp", tag="ps_small")
                nc.tensor.matmul(bp, ones1, egr, start=True, stop=True)
                nc.scalar.copy(egb[:, r, :], bp)
            whg = [whgp.tile([P, SC], BF, name=f"whg{jt}", tag=f"whg{jt}")
                   for jt in range(njt)]
            fh32 = [smallp.tile([P, 4], F32, name=f"fh{jt}", tag=f"fh{jt}")
                    for jt in range(njt)]
            for jt in range(njt):
                if layer == 1:
                    nc.sync.dma_start(out=madj_sb[jt], in_=madj[jt])
                nc.sync.dma_start(out=whg[jt], in_=src_whg(jt))
                nc.vector.tensor_copy(out=fh32[jt], in_=whg[jt][:, 260:264])
            return whg, fh32, egb

        def attention(layer, whg, fh32, egb):
            """Per-head agg psum tiles: rows 0:64 = sum alpha*WhF (i.e.
            numerator), row 64 = sum alpha*F (denominator), over all j."""
            aggps = [psum_agg.tile([P, ni], F32, name=f"agg{h}", tag=f"agg{h}")
                     for h in range(H)]
            # consume half-0 gathered tiles first (layers 2,3 arrive as two
            # half-gathers): ~half the attention runs during the second
            # half's flight instead of stalling at jt=2
            jt_order = [jt for jt in range(njt) if (jt % 4) < 2] + \
                [jt for jt in range(njt) if (jt % 4) >= 2]
            # gp heads' alpha is emitted first each jt round: the Pool
            # mask-multiply is ~2x slower than the DVE fused op, and its
            # aggregation matmul runs last, giving it maximal lead time
            head_order = sorted(range(H), key=lambda h: h not in cfg.gp_heads)
            for jn, jt in enumerate(jt_order):
                alphas = {}
                for h in head_order:
                    alpha = alphap.tile([P, ni], BF, name="alpha", tag="alpha")
                    if h in cfg.gp_heads:
                        # TensorScalarPtr (AP scalar) is not legal on Pool:
                        # split into DVE max + Pool mask-multiply
                        mx = tmp.tile([P, ni], BF, name="mx", tag="mx")
                        nc.vector.tensor_scalar_max(
                            mx, egb[:, h, :], fh32[jt][:, h:h + 1])
                        nc.gpsimd.tensor_mul(alpha, mx, madj_sb[jt])
                    else:
                        nc.vector.scalar_tensor_tensor(
                            alpha, egb[:, h, :], fh32[jt][:, h:h + 1],
                            madj_sb[jt], op0=ALU.max, op1=ALU.mult)
                    alphas[h] = alpha
                for h in range(H):
                    nc.tensor.matmul(aggps[h][0:HID + 1, :],
                                     whg[jt][:, 65 * h:65 * h + 65], alphas[h],
                                     start=(jn == 0), stop=(jn == njt - 1))
            return aggps

        def copy_aggT(aggps):
            """PSUM agg -> SBUF [65, ni] per head (transpose input must be
            SBUF; also frees the agg PSUM banks)."""
            aggT = [tmp.tile([HID + 1, ni], F32, name=f"aggT{h}", tag=f"aggT{h}")
                    for h in range(H)]
            for h in range(H):
                nc.scalar.copy(aggT[h], aggps[h][0:HID + 1, :])
            return aggT

        def norm_block(it, aggT, ht, mean_heads=False):
            """One i-block: transpose agg back to [i, f], divide by the
            denominator row, write into ht ([P, D] or [P, HID] f32)."""
            for h in range(H):
                tp = psum_sm.tile([P, P], F32, name="tpn", tag="ps_small")
                nc.tensor.transpose(
                    tp[:, 0:HID + 1],
                    aggT[h][:, it * P:(it + 1) * P],
                    ident_f[0:HID + 1, 0:HID + 1])
                rcol = smallp.tile([P, 1], F32, name="rcol", tag="rcol")
                nc.vector.reciprocal(rcol, tp[:, HID:HID + 1])
                if not mean_heads:
                    nc.vector.tensor_scalar_mul(
                        ht[:, HID * h:HID * (h + 1)], tp[:, 0:HID], rcol)
                elif h == 0:
                    nc.vector.tensor_scalar(
                        ht, tp[:, 0:HID], rcol, 1.0 / H,
                        op0=ALU.mult, op1=ALU.mult)
                else:
                    mtmp = smallp.tile([P, HID], F32, name="mtmp", tag="mtmp")
                    nc.vector.tensor_scalar(
                        mtmp, tp[:, 0:HID], rcol, 1.0 / H,
                        op0=ALU.mult, op1=ALU.mult)
                    nc.vector.tensor_add(ht, ht, mtmp)

        def layer_norm(x_t, width):
            """In-place LN over free dim (g==1, b==0)."""
            stats = smallp.tile([P, 6], F32, name="bnst", tag="bnst")
            nc.vector.bn_stats(out=stats, in_=x_t[:, 0:width])
            mv = smallp.tile([P, 2], F32, name="bnag", tag="bnag")
            nc.vector.bn_aggr(out=mv, in_=stats)
            sq = smallp.tile([P, 1], F32, name="sq", tag="sq")
            nc.scalar.activation(sq, mv[:, 1:2], AF.Sqrt, bias=eps_sb, scale=1.0)
            rstd = smallp.tile([P, 1], F32, name="rstd", tag="rstd")
            nc.vector.reciprocal(rstd, sq)
            nc.vector.tensor_scalar(
                x_t[:, 0:width], x_t[:, 0:width], mv[:, 0:1], rstd,
                op0=ALU.subtract, op1=ALU.mult)

        def elu_inplace(x_t, width, pdim=P):
            """x = elu(x) = relu(x) + exp(min(x,0)) - 1.

            All on DVE: gpsimd runs these f32 ops ~10x slower (3.8us per
            [128,256] op) and they sit on the layer-boundary critical path.
            """
            t1 = smallp.tile([pdim, width], F32, name="el1",
                             tag=f"el1_{pdim}_{width}")
            nc.vector.tensor_scalar_min(t1, x_t[:, 0:width], 0.0)
            e1 = smallp.tile([pdim, width], F32, name="el2",
                             tag=f"el2_{pdim}_{width}")
            nc.scalar.activation(e1, t1, AF.Exp, scale=1.0)
            t3 = smallp.tile([pdim, width], F32, name="el3",
                             tag=f"el3_{pdim}_{width}")
            nc.vector.tensor_scalar(t3, x_t[:, 0:width], 0.0, -1.0,
                                    op0=ALU.max, op1=ALU.add)
            nc.vector.tensor_add(x_t[:, 0:width], e1, t3)

        def emit_half_gather(hf):
            if cfg.fake_cc:
                for g in range(4):
                    nc.sync.dma_start(
                        out=cc_out[hf][g * nh:(g + 1) * nh, :],
                        in_=cc_in[hf * nh:(hf + 1) * nh, :])
            else:
                nc.gpsimd.collective_compute(
                    "AllGather", ALU.bypass, replica_groups=groups,
                    ins=[cc_in[hf * nh:(hf + 1) * nh, :]],
                    outs=[cc_out[hf][:]])

        def boundary(layer, aggps, skip_fn):
            """Fused layer boundary, pipelined per i-block: normalize ->
            skip add -> LN -> ELU -> cast/transpose -> next-layer WhF/F/q/r
            -> stage -> half-gathers. skip_fn(it, ht) adds the skip into
            ht in place. Returns (whg_src fn, egT, hT_bf, h tiles)."""
            nl = layer + 1
            aggT = copy_aggT(aggps)
            h_sb = [hp.tile([P, D], F32, name=f"h{layer}_{it}",
                            tag=f"h{layer}_{it}") for it in range(nit)]
            hTb = [stp.tile([P, ni], BF, name=f"hT{layer}_{kt}", tag=f"hT{layer}_{kt}")
                   for kt in range(2)]
            ego = [smallp.tile([P, 4], BF, name="ego", tag=f"ego{it}")
                   for it in range(nit)]
            # phase A per i-block: ACT sees only Sqrt/Exp; phase B per
            # i-block: ACT sees only Copy/Exp. Interleaving A and B puts a
            # third function in the 2-slot activation-table cache and each
            # i-block then pays ~2 1.5us ACT_TABLE_LOADs.
            for it in range(nit):
                ht = h_sb[it]
                norm_block(it, aggT, ht)
                skip_fn(it, ht)
                layer_norm(ht, D)
                elu_inplace(ht, D)
            for it in range(nit):
                ht = h_sb[it]
                hbf = smallp.tile([P, D], BF, name="hbf", tag="hbf")
                nc.vector.tensor_copy(out=hbf, in_=ht)
                for kt in range(2):
                    tp = psum_sm.tile([P, P], BF, name="tph", tag="ps_small")
                    nc.tensor.transpose(tp, hbf[:, kt * P:(kt + 1) * P],
                                        ident_bf)
                    nc.vector.tensor_copy(out=hTb[kt][:, it * P:(it + 1) * P],
                                          in_=tp)
                whp = psum_sm.tile([P, D], F32, name="whp", tag="ps_small")
                sdp = psum_sm.tile([P, 8], F32, name="sdp", tag="ps_small")
                for kt in range(2):
                    nc.tensor.matmul(whp, hTb[kt][:, it * P:(it + 1) * P],
                                     w_sb[nl][kt], start=(kt == 0), stop=(kt == 1))
                    nc.tensor.matmul(sdp, hTb[kt][:, it * P:(it + 1) * P],
                                     asd_sb[nl][kt], start=(kt == 0), stop=(kt == 1))
                st = stp.tile([P, SC], BF, name="stg", tag="stg")
                fcol = smallp.tile([P, 4], F32, name="fcol", tag="fcol")
                nc.scalar.activation(fcol, sdp[:, 4:8], AF.Exp, scale=1.0)
                dst = st[:, 0:260].rearrange("p (h c) -> p h c", c=65)
                for h in range(H):
                    nc.scalar.activation(
                        dst[:, h, 0:HID], whp[:, HID * h:HID * (h + 1)],
                        AF.Copy, scale=fcol[:, h:h + 1])
                nc.vector.tensor_copy(out=dst[:, :, HID], in_=fcol)
                nc.scalar.activation(st[:, 260:264], sdp[:, 4:8], AF.Exp,
                                     scale=-0.8)
                nc.sync.dma_start(out=cc_in[it * P:(it + 1) * P, :], in_=st)
                if it == 1:
                    # first half-gather fires as soon as it-blocks 0,1 are
                    # staged; emitted here so the in-order gpsimd queue can
                    # issue it while blocks 2,3 are still being computed
                    emit_half_gather(0)
                nc.scalar.activation(ego[it], sdp[:, 0:4], AF.Exp, scale=0.8)
            emit_half_gather(1)
            # transpose own r to row-major [4, ni]
            egT = stp.tile([4, ni], BF, name=f"egT{layer}", tag="egT")
            for it in range(nit):
                tp = psum_sm.tile([P, P], BF, name="tpe", tag="ps_small")
                nc.tensor.transpose(tp[0:4, 0:P], ego[it], ident_bf[:, 0:P])
                nc.vector.tensor_copy(out=egT[:, it * P:(it + 1) * P],
                                      in_=tp[0:4, 0:P])

            def whg_src(jt):
                g, loc = jt // 4, (jt % 4) * P
                hf, lo = (0, loc) if loc < nh else (1, loc - nh)
                return cc_out[hf][g * nh + lo:g * nh + lo + P, :]
            return whg_src, egT, hTb, h_sb

        # ============ layer 1 ============
        eg1_sb = stp.tile([4, ni], BF, name="eg1sb", tag="egT")
        nc.sync.dma_start(out=eg1_sb, in_=eg1[:])
        whg, fh32, egb = load_layer_inputs(1, lambda jt: stage1[jt], eg1_sb)
        aggps = attention(1, whg, fh32, egb)
        load_late_consts()
        xs = [smallp.tile([P, D], F32, name="xs1", tag=f"xs1_{it}")
              for it in range(nit)]
        for it in range(nit):
            nc.sync.dma_start(out=xs[it], in_=xs1[it * P:(it + 1) * P, :])

        def skip1_fn(it, ht):
            nc.vector.tensor_add(ht, ht, xs[it])

        # ============ layer 2 ============
        whg_src, egd, _hT1, h1 = boundary(1, aggps, skip1_fn)
        whg, fh32, egb = load_layer_inputs(2, whg_src, egd)
        aggps = attention(2, whg, fh32, egb)

        def skip2_fn(it, ht):
            nc.vector.tensor_add(ht, ht, h1[it])

        # ============ layer 3 ============
        whg_src, egd, hT2, h2 = boundary(2, aggps, skip2_fn)
        whg, fh32, egb = load_layer_inputs(3, whg_src, egd)
        aggps = attention(3, whg, fh32, egb)

        # mean over heads, skip3, LN(64), then head MLP in transposed
        # space: z^T = hW1^T @ h3^T, elu, out = hW2^T @ z^T + b
        aggT = copy_aggT(aggps)
        h3T = hp.tile([HID, ni], BF, name="h3T", tag="h3T")
        for it in range(nit):
            ht = hp.tile([P, HID], F32, name=f"h3_{it}", tag=f"h3_{it}")
            norm_block(it, aggT, ht, mean_heads=True)
            skp = psum_sm.tile([P, HID], F32, name="skp", tag="ps_small")
            for kt in range(2):
                nc.tensor.matmul(skp, hT2[kt][:, it * P:(it + 1) * P],
                                 skip3_sb[kt], start=(kt == 0), stop=(kt == 1))
            nc.vector.tensor_add(ht, ht, skp)
            layer_norm(ht, HID)
            if cfg.debug:
                nc.sync.dma_start(out=dbg_h3[it * P:(it + 1) * P, :], in_=ht)
            h3b = smallp.tile([P, HID], BF, name="h3b", tag="h3b")
            nc.vector.tensor_copy(out=h3b, in_=ht)
            tp = psum_sm.tile([P, P], BF, name="tp3", tag="ps_small")
            nc.tensor.transpose(tp[0:HID, 0:P], h3b, ident_bf[:, 0:P])
            nc.vector.tensor_copy(out=h3T[:, it * P:(it + 1) * P],
                                  in_=tp[0:HID, 0:P])
        zps = psum_sm.tile([32, ni], F32, name="zps", tag="ps_small")
        nc.tensor.matmul(zps, hmlp1_sb[0:HID, :], h3T, start=True, stop=True)
        zel = tmp.tile([32, ni], F32, name="zel", tag="zel")
        nc.scalar.activation(zel, zps, AF.Identity, bias=hb1c_sb, scale=1.0)
        elu_inplace(zel, ni, pdim=32)
        zbf = tmp.tile([32, ni], BF, name="zbf", tag="zbf")
        nc.vector.tensor_copy(out=zbf, in_=zel)
        ops = psum_sm.tile([1, ni], F32, name="ops", tag="ps_small")
        nc.tensor.matmul(ops, hmlp2_sb[0:32, :], zbf, start=True, stop=True)
        orow = smallp.tile([1, ni], F32, name="orow", tag="orow")
        nc.scalar.activation(orow, ops, AF.Identity,
                             bias=hmlp2_sb[32:33, 0:1], scale=1.0)
        nc.sync.dma_start(out=out_d[:, :], in_=orow)

    nc.compile()
    return nc


# =================== host side ===================

def _prep_core_inputs(inputs, cfg: Cfg, n_cores=N_CORES):
    """Build per-core in_maps from the full problem inputs."""
    x = np.asarray(inputs["x"], np.float32)
    adj = np.asarray(inputs["adj"])
    n, ni = cfg.n, cfg.ni
    f32 = np.float32

    def bf(a):
        return np.ascontiguousarray(a.astype(bf16))

    # shared weights
    def kt_split(w):  # [D, c] -> [2, 128, c]
        return np.stack([w[0:P], w[P:2 * P]])

    w2m, w3m = np.asarray(inputs["W2"], f32), np.asarray(inputs["W3"], f32)
    a2, a3 = np.asarray(inputs["a2"], f32), np.asarray(inputs["a3"], f32)

    def asd(a, W):  # s/d = (h @ W) @ selector = h @ (W @ selector)
        m = np.zeros((D, 8), f32)
        for h in range(H):
            m[h * HID:(h + 1) * HID, h] = a[h, :HID]
            m[h * HID:(h + 1) * HID, 4 + h] = a[h, HID:]
        return kt_split(W @ m)

    hmlp1 = np.concatenate([np.asarray(inputs["hW1"], f32),
                            np.asarray(inputs["hb1"], f32)[None, :]], 0)
    hmlp2 = np.concatenate([np.asarray(inputs["hW2"], f32),
                            np.asarray(inputs["hb2"], f32)[None, :]], 0)
    shared = {
        "w2": bf(kt_split(w2m)), "w3": bf(kt_split(w3m)),
        "asd2": bf(asd(a2, w2m)), "asd3": bf(asd(a3, w3m)),
        "skip3": bf(kt_split(np.asarray(inputs["skip3"], f32))),
        "hmlp1": bf(hmlp1), "hmlp2": bf(hmlp2),
        "hb1c": bf(np.asarray(inputs["hb1"], f32)[:, None]),
    }
    for gk, bk in (("g1", "b1"), ("g2", "b2"), ("g3", "b3")):
        assert np.allclose(inputs[gk], 1.0) and np.allclose(inputs[bk], 0.0), \
            "kernel built without LN affine; unexpected g/b values"

    # per-batch layer-1 precompute (shared by the 4 cores of each batch)
    batch_cache = {}
    for b in range(B):
        Wh1 = x[b] @ np.asarray(inputs["W1"], f32)            # [n, D]
        s1 = np.einsum("nhf,hf->nh", Wh1.reshape(n, H, HID),
                       np.asarray(inputs["a1"], f32)[:, :HID])
        d1 = np.einsum("nhf,hf->nh", Wh1.reshape(n, H, HID),
                       np.asarray(inputs["a1"], f32)[:, HID:])
        F1 = np.exp(d1)                                       # [n, H]
        st1 = np.zeros((cfg.njt, P, SC), f32)
        whr = (Wh1.reshape(n, H, HID) * F1[:, :, None]).reshape(
            cfg.njt, P, H, HID)
        f1r = F1.reshape(cfg.njt, P, H)
        for h in range(H):
            st1[:, :, 65 * h:65 * h + HID] = whr[:, :, h]
            st1[:, :, 65 * h + HID] = f1r[:, :, h]
        st1[:, :, 260:264] = np.exp(-0.8 * d1).reshape(cfg.njt, P, H)
        batch_cache[b] = (bf(st1), s1,
                          np.asarray(adj[b]),
                          x[b] @ np.asarray(inputs["skip1"], f32))

    in_maps = []
    for c in range(n_cores):
        b, rb = c // 4, c % 4
        sl = slice(rb * ni, (rb + 1) * ni)
        st1_bf, s1, adj_b, xs1_full = batch_cache[b]
        adjT = adj_b[sl].T.astype(f32)      # [n(src j), ni(dest)]
        im = {
            "madj": bf(adjT.reshape(cfg.njt, P, ni)),
            "stage1": st1_bf,
            "eg1": bf(np.exp(0.8 * s1[sl]).T),   # [4, ni]
            "xs1": np.ascontiguousarray(xs1_full[sl]),
            **shared,
        }
        in_maps.append(im)
    return in_maps


_CACHE = {}


def kernel(**inputs):
    cfg = Cfg()
    key = "full"
    if key not in _CACHE:
        _CACHE[key] = build_nc(cfg)
    nc = _CACHE[key]
    in_maps = _prep_core_inputs(inputs, cfg)
    from concourse.bass_utils import run_bass_kernel_spmd
    res = run_bass_kernel_spmd(nc, in_maps, list(range(N_CORES))).results
    out = np.zeros((B, N, 1), np.float32)
    ni = cfg.ni
    for c in range(N_CORES):
        b, rb = c // 4, c % 4
        out[b, rb * ni:(rb + 1) * ni] = res[c]["out"]
    return out
